# revision 1
# baseline (speedup 1.0000x reference)
"""Trainium2 Bass kernel for nn_CrossModalAttention.

Problem: bidirectional cross-attention between two (B, C, H, W) feature maps.
  B=4, C=256, H=W=64 -> N=4096 pixels, HID=64.
  For each direction:  q = Wq@xq, k = Wk@xkv, v = Wv@xkv (1x1 convs),
  attn = softmax_m(q^T k), out = xq + gamma * (v @ attn^T).

Sharding: 2 directions x 4 batches = 8 independent units, one per NeuronCore.

Per-core kernel layout trick: compute S^T tiles [m(part)=128, n(free)=512] via
matmul(lhsT=k_tile, rhs=q_tile) (contraction over HID=64 on partitions), exp on
ScalarE (logits are bounded ~ +-56, so exp in f32 needs no max-subtraction),
then accumulate U[c, n] = sum_m vT[m, c]^T expS^T[m, n] directly in PSUM across
the 32 m-blocks -- no transposes anywhere. Denominator d[n] = sum_m expS^T via
a ones[128,1] matmul accumulated in PSUM. Final: out = xq + (gamma/d)*U + gamma*bv.

Precision: S path (projections + S matmul) in float32r (TF32-like; moving dim
512 >= 256 runs at full PE rate), expS^T/vT in bf16 (measured to contribute
negligible error), all accumulation in f32 PSUM.
"""

import sys

if "/opt/trn_rl_repo" not in sys.path:
    sys.path.insert(0, "/opt/trn_rl_repo")

import ml_dtypes
import numpy as np

B = 4
C = 256
HID = 64
N = 4096          # H*W
P = 128           # SBUF partitions
NT = 512          # n-tile (matmul moving free dim)
N_NT = N // NT    # 8
MB = 128          # m-block (PV contraction tile)
N_MB = N // MB    # 32
CA = C // P       # 2 c-chunks / c-blocks

_CACHE = {}


def _build_program():
    import concourse.bass as bass
    import concourse.mybir as mybir
    from concourse import tile

    f32 = mybir.dt.float32
    f32r = mybir.dt.float32r
    bf16 = mybir.dt.bfloat16
    AF = mybir.ActivationFunctionType

    nc = bass.Bass("TRN2", target_bir_lowering=False, debug=False)

    xq_d = nc.dram_tensor("xq", (C, N), f32r, kind="ExternalInput")
    xkv_d = nc.dram_tensor("xkv", (C, N), f32r, kind="ExternalInput")
    wqT_d = nc.dram_tensor("wqT", (C, HID), f32r, kind="ExternalInput")
    wkT_d = nc.dram_tensor("wkT", (C, HID), f32r, kind="ExternalInput")
    wvT_d = nc.dram_tensor("wvT", (C, C), f32r, kind="ExternalInput")
    bq_d = nc.dram_tensor("bq", (HID, 1), f32, kind="ExternalInput")
    bk_d = nc.dram_tensor("bk", (HID, 1), f32, kind="ExternalInput")
    gbv_d = nc.dram_tensor("gbv", (C, 1), f32, kind="ExternalInput")      # gamma * bv
    rgam_d = nc.dram_tensor("rgam", (1, 1), f32, kind="ExternalInput")    # 1 / gamma
    onesr_d = nc.dram_tensor("onesr", (1, P), f32r, kind="ExternalInput")  # all-ones
    x2tb_d = nc.dram_tensor("x2tb", (N, C), bf16, kind="ExternalInput")   # bf16 xkv^T
    out_d = nc.dram_tensor("out", (C, N), f32, kind="ExternalOutput")

    # c = a*128 + p views
    xq_r = xq_d[:].rearrange("(a p) n -> p a n", p=P)
    xkv_r = xkv_d[:].rearrange("(a p) n -> p a n", p=P)
    wqT_r = wqT_d[:].rearrange("(a p) h -> p a h", p=P)
    wkT_r = wkT_d[:].rearrange("(a p) h -> p a h", p=P)
    wvT_r = wvT_d[:].rearrange("(a p) c -> p a c", p=P)
    gbv_r = gbv_d[:].rearrange("(a p) one -> p (a one)", p=P)
    out_r = out_d[:].rearrange("(a p) n -> p a n", p=P)

    with tile.TileContext(nc) as tc:
        with (
            tc.tile_pool(name="const", bufs=1) as const,
            tc.tile_pool(name="xin", bufs=1) as xin,
            tc.tile_pool(name="qk", bufs=1) as qk,
            tc.tile_pool(name="vtp", bufs=1) as vtp,
            tc.tile_pool(name="work", bufs=3) as work,
            tc.tile_pool(name="ep", bufs=2) as ep,
            tc.tile_pool(name="dram", bufs=2, space="DRAM") as dram,
            tc.tile_pool(name="psum", bufs=1, space="PSUM") as psum,
        ):
            # ---- constants / weights ---- (ACT HWDGE queue: issue-parallel
            # with the x loads on the SP queue; wk first for the k-proj)
            wk_sb = const.tile([P, CA, HID], f32r, tag="wk")
            nc.scalar.dma_start(wk_sb[:], wkT_r)
            wq_sb = const.tile([P, CA, HID], f32r, tag="wq")
            nc.scalar.dma_start(wq_sb[:], wqT_r)
            wv_sb = const.tile([P, CA, C], f32r, tag="wv")
            nc.scalar.dma_start(wv_sb[:], wvT_r)
            bq_sb = const.tile([HID, 1], f32, tag="bq")
            nc.scalar.dma_start(bq_sb[:], bq_d[:])
            bk_sb = const.tile([HID, 1], f32, tag="bk")
            nc.scalar.dma_start(bk_sb[:], bk_d[:])
            gbv_sb = const.tile([P, CA], f32, tag="gbv")
            nc.scalar.dma_start(gbv_sb[:], gbv_r)
            rgam_sb = const.tile([1, 1], f32, tag="rgam")
            nc.scalar.dma_start(rgam_sb[:], rgam_d[:])
            ones_sb = const.tile([P, 1], bf16, tag="ones")
            nc.vector.memset(ones_sb[:], 1.0)
            onesr_sb = const.tile([1, P], f32r, tag="onesr")
            nc.scalar.dma_start(onesr_sb[:], onesr_d[:])

            # ---- x loads (chunked for DMA/compute overlap) ----
            xq_sb = xin.tile([P, CA, N], f32r, tag="xq")
            xkv_sb = xin.tile([P, CA, N], f32r, tag="xkv")
            NCH = 1024
            for h in range(N // NCH):
                sl = slice(h * NCH, (h + 1) * NCH)
                for a in range(CA):
                    nc.sync.dma_start(xkv_sb[:, a, sl], xkv_r[:, a, sl])
            for a in range(CA):   # first xq chunk: unblocks q-proj for nt=0
                nc.sync.dma_start(xq_sb[:, a, 0:NCH], xq_r[:, a, 0:NCH])

            # ---- projections ----
            # q/k stored twice (rows 0-63 and 64-127) so the K=64 S-matmuls
            # can be row-paired into both halves of the PE array.
            q_sb = qk.tile([P, N], f32r, tag="q")
            k_sb = qk.tile([P, N], f32r, tag="k")
            # k projection first: it only needs xkv, which is DMA'd first
            for nt in range(N_NT):
                ntsl = slice(nt * NT, (nt + 1) * NT)
                kp = psum.tile([P, NT], f32, tag="st", bufs=3)
                for a in range(CA):
                    nc.tensor.matmul(
                        kp[:HID, :],
                        lhsT=wk_sb[:, a, :],
                        rhs=xkv_sb[:, a, ntsl],
                        start=(a == 0),
                        stop=(a == CA - 1),
                    )
                nc.vector.tensor_scalar_add(k_sb[0:HID, ntsl], kp[:HID, :], bk_sb[:])
                nc.vector.tensor_scalar_add(k_sb[HID:P, ntsl], kp[:HID, :], bk_sb[:])

            # X2^T tiles [m, c_in] (bf16) for the Y = X2 @ E matmuls; the
            # transpose+cast happens on the host. The v projection is folded
            # in AFTER the attention sum via associativity:
            # U = Wv @ (X2 @ E) -- saves the whole vT projection pass.
            x2t_sb = vtp.tile([P, N_MB, C], bf16, tag="x2t")
            x2t_r = x2tb_d[:].rearrange("(mb p) c -> p mb c", p=P)
            for mb in range(N_MB):
                nc.gpsimd.dma_start(x2t_sb[:, mb, :], x2t_r[:, mb, :])
            for h in range(1, N // NCH):
                sl = slice(h * NCH, (h + 1) * NCH)
                for a in range(CA):
                    nc.sync.dma_start(xq_sb[:, a, sl], xq_r[:, a, sl])

            # q projection, emitted per n-tile: nt=0 up front, nt+1 during
            # the attention m-loop of nt (so the PE stream never stalls on
            # late xq DMA chunks)
            def _qproj(nt):
                ntsl = slice(nt * NT, (nt + 1) * NT)
                qp = psum.tile([P, NT], f32, tag="st", bufs=3, name=f"qp_{nt}")
                for a in range(CA):
                    nc.tensor.matmul(
                        qp[:HID, :],
                        lhsT=wq_sb[:, a, :],
                        rhs=xq_sb[:, a, ntsl],
                        start=(a == 0),
                        stop=(a == CA - 1),
                    )
                nc.vector.tensor_scalar_add(q_sb[0:HID, ntsl], qp[:HID, :], bq_sb[:])
                nc.vector.tensor_scalar_add(q_sb[HID:P, ntsl], qp[:HID, :], bq_sb[:])

            _qproj(0)

            # ---- attention ----
            DG = 8           # m-blocks per denominator group
            N_DG = N_MB // DG

            def _epilogue_a(nt, y0, y1, dp, final_dmm):
                # finish d, grd = gamma/d on ACT+DVE, kick off the partition-
                # broadcast DRAM roundtrip, and move Y to SBUF (no PE work)
                final_dmm()
                rd = ep.tile([1, NT], f32, tag="rd", name=f"rd_{nt}")
                nc.scalar.activation(rd[:], dp[:], AF.Copy, scale=rgam_sb[:])
                grd = ep.tile([1, NT], f32, tag="grd", name=f"grd_{nt}")
                nc.vector.reciprocal(grd[:], rd[:])
                dscr = dram.tile([1, NT], f32, tag="dscr", name=f"dscr_{nt}")
                nc.sync.dma_start(dscr[:], grd[:])
                rdb = ep.tile([P, NT], f32, tag="rdb", name=f"rdb_{nt}")
                nc.sync.dma_start(rdb[:], dscr[:].broadcast_to((P, NT)))
                yb0 = ep.tile([P, NT], f32r, tag="yb0", name=f"yb0_{nt}")
                nc.scalar.copy(yb0[:], y0[:])
                yb1 = ep.tile([P, NT], f32r, tag="yb1", name=f"yb1_{nt}")
                nc.scalar.copy(yb1[:], y1[:])
                return rdb, yb0, yb1

            def _epilogue_b(nt, state):
                # U = Wv @ Y (f32r, 2 accumulating matmuls per c-block), then
                # out[c, n] = xq + rdb[n] * U[c, n] + gamma*bv[c]
                rdb, yb0, yb1 = state
                ntsl = slice(nt * NT, (nt + 1) * NT)
                for cb in range(CA):
                    ups = psum.tile(
                        [P, NT], f32, tag="st", bufs=3, name=f"ups_{nt}_{cb}"
                    )
                    nc.tensor.matmul(
                        ups[:], lhsT=wv_sb[:, 0, cb * P : (cb + 1) * P],
                        rhs=yb0[:], start=True, stop=False,
                    )
                    nc.tensor.matmul(
                        ups[:], lhsT=wv_sb[:, 1, cb * P : (cb + 1) * P],
                        rhs=yb1[:], start=False, stop=True,
                    )
                    t = ep.tile([P, NT], f32, tag="t", name=f"t_{nt}_{cb}")
                    nc.vector.tensor_mul(t[:], ups[:], rdb[:])
                    o = ep.tile([P, NT], f32, tag="o", name=f"o_{nt}_{cb}")
                    nc.vector.scalar_tensor_tensor(
                        o[:],
                        in0=t[:],
                        scalar=gbv_sb[:, cb : cb + 1],
                        in1=xq_sb[:, cb, ntsl],
                        op0=mybir.AluOpType.add,
                        op1=mybir.AluOpType.add,
                    )
                    nc.sync.dma_start(out_r[:, cb, ntsl], o[:])

            # previous n-tile's epilogue stages, deferred into the next
            # n-tile's m-loop so they never stall the in-order PE queue
            pending_a = [None]
            pending_b = [None]

            for nt in range(N_NT):
                ntsl = slice(nt * NT, (nt + 1) * NT)
                y0 = psum.tile([P, NT], f32, tag="y", bufs=4, name=f"y0_{nt}")
                y1 = psum.tile([P, NT], f32, tag="y", bufs=4, name=f"y1_{nt}")
                dp = psum.tile([1, NT], f32, tag="dd", bufs=1, name=f"dp_{nt}")
                acc = None   # running bf16 partial-sum for the current d group
                n_d = 0      # d-matmuls issued for this n-tile
                for mb in range(N_MB):
                    msl = slice(mb * MB, (mb + 1) * MB)
                    # row-paired S matmul: even m-blocks use PE rows 0-63,
                    # odd ones rows 64-127 (concurrent via tile_position)
                    half = slice(0, HID) if mb % 2 == 0 else slice(HID, P)
                    stp = psum.tile([P, NT], f32, tag="st", bufs=3, name=f"stp_{nt}_{mb}")
                    nc.tensor.matmul(
                        stp[:],
                        lhsT=k_sb[half, msl],
                        rhs=q_sb[half, ntsl],
                        start=True,
                        stop=True,
                    )
                    ex = work.tile([P, NT], bf16, tag="expst", name=f"ex_{nt}_{mb}")
                    nc.scalar.activation(ex[:], stp[:], AF.Exp)
                    first, last = (mb == 0), (mb == N_MB - 1)
                    nc.tensor.matmul(
                        y0[:], lhsT=x2t_sb[:, mb, 0:P], rhs=ex[:], start=first, stop=last
                    )
                    nc.tensor.matmul(
                        y1[:], lhsT=x2t_sb[:, mb, P:C], rhs=ex[:], start=first, stop=last
                    )
                    if mb == 1 and nt + 1 < N_NT:
                        _qproj(nt + 1)
                    if mb == 3 and pending_a[0] is not None:
                        grd_prev = pending_a[0]()
                        pending_a[0] = None
                        pb = pending_b[0]
                        pending_b[0] = lambda grd_prev=grd_prev, pb=pb: pb(grd_prev)
                    if mb == 12 and pending_b[0] is not None:
                        pending_b[0]()
                        pending_b[0] = None
                    # denominator: running bf16 sum on DVE; one ones-matmul
                    # per DG m-blocks accumulated into dp
                    if mb % DG == 0:
                        acc = ex
                    else:
                        s_ = work.tile(
                            [P, NT], bf16, tag=f"dacc{mb % 2}", bufs=3,
                            name=f"ds_{nt}_{mb}",
                        )
                        nc.vector.tensor_add(s_[:], acc[:], ex[:])
                        acc = s_
                    if (mb + 1) % DG == 0:
                        n_d += 1
                        a8, nd = acc, n_d
                        def _dmm(a8=a8, nd=nd, dp=dp):
                            nc.tensor.matmul(
                                dp[:], lhsT=ones_sb[:], rhs=a8[:],
                                start=(nd == 1), stop=(nd == N_DG),
                            )
                        if nd == N_DG:
                            pending_a[0] = (
                                lambda nt=nt, y0=y0, y1=y1, dp=dp, dmm=_dmm:
                                _epilogue_a(nt, y0, y1, dp, dmm)
                            )
                            pending_b[0] = (
                                lambda state, nt=nt: _epilogue_b(nt, state)
                            )
                        else:
                            _dmm()
                        acc = None
            state_last = pending_a[0]()
            pending_b[0](state_last)

    return nc


def _split_excess_waits(nc):
    """The pinned walrus build only encodes 1 sync-wait per instruction;
    newer concourse attaches more. Hoist excess waits onto same-engine NoOps
    inserted immediately before the over-limit instruction (semantically
    identical: same engine, same program position)."""
    import concourse.mybir as mybir
    import bass_rust

    ctr = 0
    for bbl in nc.m.functions[0].blocks:
        il = bbl.instructions
        i = 0
        while i < len(il):
            inst = il[i]
            si = inst.sync_info
            limit = 1
            if si is not None and len(si.on_wait) > limit:
                waits = list(si.on_wait)
                extra = waits[limit:]
                for j in range(0, len(extra), 1):
                    nop = mybir.InstNoOp(name=f"I-wsplit-{ctr}", ins=[], outs=[])
                    ctr += 1
                    nop.engine = inst.engine
                    nop.sync_info = bass_rust.SyncInfo(
                        on_wait=[extra[j]], on_update=[]
                    )
                    il.insert(i, nop)
                    i += 1
                si.on_wait = waits[:limit]
                inst.sync_info = si
            i += 1
    return ctr


def _get_program():
    if "nc" not in _CACHE:
        _CACHE["nc"] = _build_program()
    return _CACHE["nc"]


def _get_program_hw():
    """Program with the walrus sync-wait workaround applied (breaks CoreSim's
    race detector, so only applied for hardware runs)."""
    nc = _get_program()
    if not _CACHE.get("split_done"):
        _split_excess_waits(nc)
        _CACHE["split_done"] = True
    return nc


def _make_in_maps(x1, x2, Wq, bq, Wk, bk, Wv, bv, gamma):
    g = float(np.asarray(gamma).reshape(-1)[0])
    shared = {
        "wqT": np.ascontiguousarray(Wq.T, dtype=np.float32),
        "wkT": np.ascontiguousarray(Wk.T, dtype=np.float32),
        "wvT": np.ascontiguousarray(Wv.T, dtype=np.float32),
        "bq": np.asarray(bq, dtype=np.float32).reshape(HID, 1),
        "bk": np.asarray(bk, dtype=np.float32).reshape(HID, 1),
        "gbv": (g * np.asarray(bv, dtype=np.float32)).reshape(C, 1),
        "rgam": np.array([[1.0 / g if g != 0.0 else 0.0]], dtype=np.float32),
        "onesr": np.ones((1, 128), dtype=np.float32),
    }
    in_maps = []
    for d in range(2):
        src_q, src_kv = (x1, x2) if d == 0 else (x2, x1)
        for b in range(B):
            xkv_f32 = np.ascontiguousarray(src_kv[b].reshape(C, N), dtype=np.float32)
            in_maps.append(
                {
                    "xq": np.ascontiguousarray(src_q[b].reshape(C, N), dtype=np.float32),
                    "xkv": xkv_f32,
                    "x2tb": np.ascontiguousarray(xkv_f32.T).astype(ml_dtypes.bfloat16),
                    **shared,
                }
            )
    return in_maps


def kernel(x1, x2, Wq, bq, Wk, bk, Wv, bv, gamma, _want_results=False):
    x1 = np.asarray(x1, dtype=np.float32)
    x2 = np.asarray(x2, dtype=np.float32)
    nc = _get_program_hw()
    in_maps = _make_in_maps(x1, x2, Wq, bq, Wk, bk, Wv, bv, gamma)

    from concourse.bass_utils import run_bass_kernel_spmd

    res = run_bass_kernel_spmd(nc, in_maps, core_ids=list(range(2 * B)))
    outs = [r["out"].reshape(C, 64, 64) for r in res.results]
    out1 = np.stack(outs[:B]).astype(np.float32)
    out2 = np.stack(outs[B:]).astype(np.float32)
    if _want_results:
        return (out1, out2), res
    return (out1, out2)



# revision 3
# speedup vs baseline: 1.3823x; 1.3823x over previous
"""Trainium2 Bass kernel for nn_CrossModalAttention.

Problem: bidirectional cross-attention between two (B, C, H, W) feature maps.
  B=4, C=256, H=W=64 -> N=4096 pixels, HID=64.
  For each direction:  q = Wq@xq, k = Wk@xkv, v = Wv@xkv (1x1 convs),
  attn = softmax_m(q^T k), out = xq + gamma * (v @ attn^T).

Sharding: 2 directions x 4 batches = 8 independent units, one per NeuronCore.

Per-core layout: compute S^T tiles [m(part)=128, n(free)=512] via
matmul(lhsT=k_tile, rhs=q_tile) (contraction over HID=64 on partitions).
All matmul operands are bf16: fp32 operands stream through the PE at half
rate (fp32_mode=HIGH measured 501ns vs 226ns for N=512), and numpy
simulation of the full bf16 pipeline gives rel err ~5e-3 (budget 2e-2).

Key scheduling facts (measured on HW):
  - K=64 S-matmuls issued back-to-back with lhsT base partitions 0/64
    auto-derive tile_position row groups and run CONCURRENTLY (dstart
    ~3ns).  So the even/odd m-block S matmuls are emitted adjacently.
  - The PE queue is in-order: PV (which waits on exp) must not sit
    between S matmuls, so S(j+1) is issued before PV(j) (software
    pipelining) to keep the PE streaming.
  - ACT costs (N+352)/1.2 ns per ACTIVATE: exp is done on [128,1024]
    2-PSUM-bank pair tiles to amortize the fixed overhead.
  - HAM clock: the PE runs at 1.2 GHz until ~3.4us of sustained busy;
    a warm-up burst of small matmuls runs during the initial DMAs.

The v projection is folded AFTER the attention sum via associativity:
U = Wv @ (X2 @ E) -- saves nothing in FLOPs but avoids materializing V
and lets X2^T (host-transposed, bf16) be the PV weight operand.

Softmax algebra: per-n additive logit terms cancel between numerator and
denominator, so q/k biases reduce to a per-m term a_m = bq.(Wk xkv)_m,
handled (only when nonzero; setup_inputs uses zero biases) by per-half
exp bias tiles and host-scaled x2tb rows.  gamma and bv are host-folded
(gbv = gamma*bv, gam = gamma broadcast).

Denominator: running bf16 sums of exp pair-tiles on DVE (2 groups of 8
pairs per n-tile), reduced across partitions by ones-matmuls into PSUM.
Reciprocal runs on a [128,4] reshape (DRAM roundtrip) instead of [1,512]
(DVE reciprocal is 8 cycles/elem *per lane*; [1,512] wastes 127 lanes).
"""

import sys

if "/opt/trn_rl_repo" not in sys.path:
    sys.path.insert(0, "/opt/trn_rl_repo")

import ml_dtypes
import numpy as np

B = 4
C = 256
HID = 64
N = 4096          # H*W
P = 128           # SBUF partitions
NT = 512          # n-tile (matmul moving free dim)
N_NT = N // NT    # 8
MB = 128          # m-block (PV contraction tile)
N_MB = N // MB    # 32
NPAIR = N_MB // 2 # 16 m-block pairs per n-tile
CA = C // P       # 2 c-chunks / c-blocks
NCH = 512         # x DMA chunk (columns)
DG = 8            # pair-steps per denominator group (2 groups per n-tile)
WARM_MMS = 44     # PE warm-up burst size

_CACHE = {}


def _build_program(with_qk_bias=False):
    import concourse.bass as bass
    import concourse.mybir as mybir
    from concourse import tile

    f32 = mybir.dt.float32
    bf16 = mybir.dt.bfloat16
    AF = mybir.ActivationFunctionType

    nc = bass.Bass("TRN2", target_bir_lowering=False, debug=False)

    xqb_d = nc.dram_tensor("xqb", (C, N), bf16, kind="ExternalInput")
    xkvb_d = nc.dram_tensor("xkvb", (C, N), bf16, kind="ExternalInput")
    x2tb_d = nc.dram_tensor("x2tb", (N, C), bf16, kind="ExternalInput")
    wqT_d = nc.dram_tensor("wqT", (C, HID), bf16, kind="ExternalInput")
    wkT_d = nc.dram_tensor("wkT", (C, HID), bf16, kind="ExternalInput")
    wvT_d = nc.dram_tensor("wvT", (C, C), bf16, kind="ExternalInput")
    gbv_d = nc.dram_tensor("gbv", (C, 1), f32, kind="ExternalInput")   # gamma*bv
    gam_d = nc.dram_tensor("gam", (P, 1), f32, kind="ExternalInput")   # gamma bcast
    if with_qk_bias:
        # a32[p, mb] = bq . k_raw[:, mb*128+p]  (exp bias, per-m term)
        a32_d = nc.dram_tensor("a32", (P, N_MB), f32, kind="ExternalInput")
    out_d = nc.dram_tensor("out", (C, N), f32, kind="ExternalOutput")

    # c = a*128 + p views
    xqb_r = xqb_d[:].rearrange("(a p) n -> p a n", p=P)
    xkvb_r = xkvb_d[:].rearrange("(a p) n -> p a n", p=P)
    x2t_r = x2tb_d[:].rearrange("(mb p) c -> p mb c", p=P)
    wqT_r = wqT_d[:].rearrange("(a p) h -> p a h", p=P)
    wkT_r = wkT_d[:].rearrange("(a p) h -> p a h", p=P)
    wvT_r = wvT_d[:].rearrange("(a p) c -> p a c", p=P)
    gbv_r = gbv_d[:].rearrange("(a p) one -> p (a one)", p=P)
    out_r = out_d[:].rearrange("(a p) n -> p a n", p=P)

    with tile.TileContext(nc) as tc:
        with (
            tc.tile_pool(name="const", bufs=1) as const,
            tc.tile_pool(name="xin", bufs=1) as xin,
            tc.tile_pool(name="vtp", bufs=1) as vtp,
            tc.tile_pool(name="qk", bufs=1) as qk,
            tc.tile_pool(name="work", bufs=3) as work,
            tc.tile_pool(name="ep", bufs=2) as ep,
            tc.tile_pool(name="dram", bufs=2, space="DRAM") as dram,
            tc.tile_pool(name="psum", bufs=1, space="PSUM") as psum,
        ):
            # ---- constants / weights (ACT HWDGE queue) ----
            wk_sb = const.tile([P, CA, HID], bf16, tag="wk")
            nc.scalar.dma_start(wk_sb[:], wkT_r)
            wq_sb = const.tile([P, CA, HID], bf16, tag="wq")
            nc.scalar.dma_start(wq_sb[:], wqT_r)
            wv_sb = const.tile([P, CA, C], bf16, tag="wv")
            nc.scalar.dma_start(wv_sb[:], wvT_r)
            gbv_sb = const.tile([P, CA], f32, tag="gbv")
            nc.scalar.dma_start(gbv_sb[:], gbv_r)
            gam_sb = const.tile([P, 1], f32, tag="gam")
            nc.scalar.dma_start(gam_sb[:], gam_d[:])
            if with_qk_bias:
                a32_sb = const.tile([P, N_MB], f32, tag="a32")
                nc.scalar.dma_start(a32_sb[:], a32_d[:])
            ones_sb = const.tile([P, 1], bf16, tag="ones")
            nc.vector.memset(ones_sb[:], 1.0)
            warm_sb = const.tile([P, 64], bf16, tag="warm")
            nc.vector.memset(warm_sb[:], 0.0)

            # ---- x loads (SP queue, chunked for DMA/compute overlap) ----
            xqb_sb = xin.tile([P, CA, N], bf16, tag="xqb")
            xkvb_sb = xin.tile([P, CA, N], bf16, tag="xkvb")
            x2t_sb = vtp.tile([P, N_MB, C], bf16, tag="x2t")

            def _ldx(dst, src, ch):
                sl = slice(ch * NCH, (ch + 1) * NCH)
                for a in range(CA):
                    nc.sync.dma_start(dst[:, a, sl], src[:, a, sl])

            _ldx(xkvb_sb, xkvb_r, 0)
            _ldx(xqb_sb, xqb_r, 0)
            for ch in range(1, N // NCH):
                _ldx(xkvb_sb, xkvb_r, ch)
            for ch in range(1, N // NCH):
                _ldx(xqb_sb, xqb_r, ch)
            # x2^T tiles on the gpsimd HWDGE queue (runs in parallel)
            for mb in range(N_MB):
                nc.gpsimd.dma_start(x2t_sb[:, mb, :], x2t_r[:, mb, :])

            # ---- PE warm-up burst (HAM un-throttle during initial DMAs) ----
            scrW = psum.tile([P, NT], f32, tag="scr", bufs=1, name="warmps")
            for i in range(WARM_MMS):
                nc.tensor.matmul(
                    scrW[0:64, 0:64], lhsT=warm_sb[:, 0:64], rhs=warm_sb[:, 0:64],
                    start=True, stop=True,
                )

            # ---- projections (all bf16; PSUM -> SBUF dup copies on DVE) ----
            q_sb = qk.tile([P, N], bf16, tag="q")
            k_sb = qk.tile([P, N], bf16, tag="k")

            def _kproj(ch):
                sl = slice(ch * NCH, (ch + 1) * NCH)
                kp = psum.tile([P, NT], f32, tag="scr", bufs=1, name=f"kp_{ch}")
                for a in range(CA):
                    nc.tensor.matmul(
                        kp[:HID, :], lhsT=wk_sb[:, a, :], rhs=xkvb_sb[:, a, sl],
                        start=(a == 0), stop=(a == CA - 1),
                    )
                nc.vector.tensor_copy(k_sb[0:HID, sl], kp[:HID, :])
                nc.vector.tensor_copy(k_sb[HID:P, sl], kp[:HID, :])

            def _qproj(nt):
                sl = slice(nt * NT, (nt + 1) * NT)
                qp = psum.tile([P, NT], f32, tag="scr", bufs=1, name=f"qp_{nt}")
                for a in range(CA):
                    nc.tensor.matmul(
                        qp[:HID, :], lhsT=wq_sb[:, a, :], rhs=xqb_sb[:, a, sl],
                        start=(a == 0), stop=(a == CA - 1),
                    )
                nc.vector.tensor_copy(q_sb[0:HID, sl], qp[:HID, :])
                nc.vector.tensor_copy(q_sb[HID:P, sl], qp[:HID, :])

            _kproj(0)
            _qproj(0)

            # ---- attention ----
            def _spair(nt, j, name):
                """Adjacent even/odd S matmuls -> concurrent PE row halves."""
                ntsl = slice(nt * NT, (nt + 1) * NT)
                me = slice((2 * j) * MB, (2 * j) * MB + MB)
                mo = slice((2 * j + 1) * MB, (2 * j + 1) * MB + MB)
                sp = psum.tile([P, 2 * NT], f32, tag="st2", bufs=2, name=name)
                nc.tensor.matmul(
                    sp[:, 0:NT], lhsT=k_sb[0:HID, me], rhs=q_sb[0:HID, ntsl],
                    start=True, stop=True,
                )
                nc.tensor.matmul(
                    sp[:, NT:], lhsT=k_sb[HID:P, mo], rhs=q_sb[HID:P, ntsl],
                    start=True, stop=True,
                )
                return sp

            def _exp(sp, nt, j):
                ex = work.tile([P, 2 * NT], bf16, tag="ex", name=f"ex_{nt}_{j}")
                if with_qk_bias:
                    nc.scalar.activation(
                        ex[:, 0:NT], sp[:, 0:NT], AF.Exp,
                        bias=a32_sb[:, 2 * j : 2 * j + 1],
                    )
                    nc.scalar.activation(
                        ex[:, NT:], sp[:, NT:], AF.Exp,
                        bias=a32_sb[:, 2 * j + 1 : 2 * j + 2],
                    )
                else:
                    nc.scalar.activation(ex[:], sp[:], AF.Exp)
                return ex

            def _epilogue_a(nt, y0, y1, dp):
                """Free the y PSUM banks (bf16 copies) and launch the
                denominator reciprocal chain ([128,4] reshape roundtrip)."""
                yb0 = ep.tile([P, NT], bf16, tag="yb0", name=f"yb0_{nt}")
                nc.vector.tensor_copy(yb0[:], y0[:])
                yb1 = ep.tile([P, NT], bf16, tag="yb1", name=f"yb1_{nt}")
                nc.vector.tensor_copy(yb1[:], y1[:])
                rds = ep.tile([1, NT], f32, tag="rds", name=f"rds_{nt}")
                nc.vector.tensor_copy(rds[:], dp[:])
                dscr = dram.tile([1, NT], f32, tag="dscr", name=f"dscr_{nt}")
                nc.gpsimd.dma_start(dscr[:], rds[:])
                dv4 = ep.tile([P, 4], f32, tag="dv4", name=f"dv4_{nt}")
                nc.gpsimd.dma_start(
                    dv4[:], dscr[:].rearrange("o (p f) -> (o p) f", p=P)
                )
                rv4 = ep.tile([P, 4], f32, tag="rv4", name=f"rv4_{nt}")
                nc.vector.reciprocal(rv4[:], dv4[:])
                rv4g = ep.tile([P, 4], f32, tag="rv4g", name=f"rv4g_{nt}")
                nc.vector.tensor_scalar_mul(rv4g[:], rv4[:], gam_sb[:, 0:1])
                dsc2 = dram.tile([1, NT], f32, tag="dsc2", name=f"dsc2_{nt}")
                nc.gpsimd.dma_start(
                    dsc2[:].rearrange("o (p f) -> (o p) f", p=P), rv4g[:]
                )
                rdb = ep.tile([P, NT], f32, tag="rdb", name=f"rdb_{nt}")
                nc.gpsimd.dma_start(rdb[:], dsc2[:].broadcast_to((P, NT)))
                return yb0, yb1, rdb

            def _epilogue_b(nt, state, cb):
                """U = Wv @ Y (one c-block), out = xq + rdb*U + gamma*bv."""
                yb0, yb1, rdb = state
                ntsl = slice(nt * NT, (nt + 1) * NT)
                ups = psum.tile(
                    [P, NT], f32, tag="scr", bufs=1, name=f"ups_{nt}_{cb}"
                )
                nc.tensor.matmul(
                    ups[:], lhsT=wv_sb[:, 0, cb * P : (cb + 1) * P],
                    rhs=yb0[:], start=True, stop=False,
                )
                nc.tensor.matmul(
                    ups[:], lhsT=wv_sb[:, 1, cb * P : (cb + 1) * P],
                    rhs=yb1[:], start=False, stop=True,
                )
                t = ep.tile([P, NT], f32, tag="t", name=f"t_{nt}_{cb}")
                nc.vector.tensor_mul(t[:], ups[:], rdb[:])
                o = ep.tile([P, NT], f32, tag="o", name=f"o_{nt}_{cb}")
                nc.vector.scalar_tensor_tensor(
                    o[:],
                    in0=t[:],
                    scalar=gbv_sb[:, cb : cb + 1],
                    in1=xqb_sb[:, cb, ntsl],
                    op0=mybir.AluOpType.add,
                    op1=mybir.AluOpType.add,
                )
                nc.sync.dma_start(out_r[:, cb, ntsl], o[:])

            prev = None          # (nt-1)'s (y0, y1, dp) awaiting epilogue
            state = None         # epilogue_a output for (nt-1)
            pend_dp = None       # deferred final dp matmul pair of prev nt
            sp_next = None       # software-pipelined S pair tile
            ex_next = None

            for nt in range(N_NT):
                ntsl = slice(nt * NT, (nt + 1) * NT)
                y0 = psum.tile([P, NT], f32, tag="y", bufs=2, name=f"y0_{nt}")
                y1 = psum.tile([P, NT], f32, tag="y", bufs=2, name=f"y1_{nt}")
                dp = psum.tile([1, NT], f32, tag="dd", bufs=1, name=f"dp_{nt}")

                if nt == 0:
                    sp_next = _spair(0, 0, "sp_0_0")
                    ex_next = _exp(sp_next, 0, 0)
                else:
                    # deferred final dp matmuls of prev nt, then its epilogue
                    # head: free y banks, launch the reciprocal chain
                    if pend_dp is not None:
                        pend_dp()
                        pend_dp = None
                    state = _epilogue_a(nt - 1, *prev)

                acc = None
                for j in range(NPAIR):
                    sp, ex = sp_next, ex_next
                    # next S pair (concurrent row halves), ahead of PV(j)
                    if j + 1 < NPAIR:
                        sp_next = _spair(nt, j + 1, f"sp_{nt}_{j+1}")
                    elif nt + 1 < N_NT:
                        sp_next = _spair(nt + 1, 0, f"sp_{nt+1}_0")
                    else:
                        sp_next = None
                    # PV: 4 accumulating matmuls (2 m-blocks x 2 c-blocks)
                    first, last = (j == 0), (j == NPAIR - 1)
                    nc.tensor.matmul(
                        y0[:], lhsT=x2t_sb[:, 2 * j, 0:P], rhs=ex[:, 0:NT],
                        start=first, stop=False,
                    )
                    nc.tensor.matmul(
                        y1[:], lhsT=x2t_sb[:, 2 * j, P:C], rhs=ex[:, 0:NT],
                        start=first, stop=False,
                    )
                    nc.tensor.matmul(
                        y0[:], lhsT=x2t_sb[:, 2 * j + 1, 0:P], rhs=ex[:, NT:],
                        start=False, stop=last,
                    )
                    nc.tensor.matmul(
                        y1[:], lhsT=x2t_sb[:, 2 * j + 1, P:C], rhs=ex[:, NT:],
                        start=False, stop=last,
                    )
                    if sp_next is not None:
                        ex_next = _exp(
                            sp_next,
                            nt if j + 1 < NPAIR else nt + 1,
                            (j + 1) % NPAIR,
                        )
                    # interleaved projections / k chunks / prev-nt epilogues
                    if nt == 0 and j < 14 and j % 2 == 0:
                        _kproj(j // 2 + 1)
                    if j == 1 and nt + 1 < N_NT:
                        _qproj(nt + 1)
                    if j == 8 and state is not None:
                        _epilogue_b(nt - 1, state, 0)
                    if j == 10 and state is not None:
                        _epilogue_b(nt - 1, state, 1)
                        state = None
                    # denominator: running bf16 pair-tile sums on DVE, one
                    # ones-matmul pair per DG pair-steps accumulated into dp
                    if acc is None:
                        acc = ex
                    else:
                        s_ = work.tile(
                            [P, 2 * NT], bf16, tag="dacc", bufs=2,
                            name=f"da_{nt}_{j}",
                        )
                        nc.vector.tensor_add(s_[:], acc[:], ex[:])
                        acc = s_
                    if (j + 1) % DG == 0:
                        g = j // DG
                        a8 = acc

                        def _dmm(a8=a8, g=g, dp=dp):
                            nc.tensor.matmul(
                                dp[:], lhsT=ones_sb[:], rhs=a8[:, 0:NT],
                                start=(g == 0), stop=False,
                            )
                            nc.tensor.matmul(
                                dp[:], lhsT=ones_sb[:], rhs=a8[:, NT:],
                                start=False, stop=(g == NPAIR // DG - 1),
                            )

                        if g == NPAIR // DG - 1:
                            pend_dp = _dmm   # run at next nt's first iter
                        else:
                            _dmm()
                        acc = None
                prev = (y0, y1, dp)

            # drain: last n-tile's epilogue
            if pend_dp is not None:
                pend_dp()
                pend_dp = None
            state = _epilogue_a(N_NT - 1, *prev)
            _epilogue_b(N_NT - 1, state, 0)
            _epilogue_b(N_NT - 1, state, 1)

    return nc


def _split_excess_waits(nc):
    """The pinned walrus build only encodes 1 sync-wait per instruction;
    newer concourse attaches more. Hoist excess waits onto same-engine NoOps
    inserted immediately before the over-limit instruction (semantically
    identical: same engine, same program position)."""
    import concourse.mybir as mybir
    import bass_rust

    ctr = 0
    for bbl in nc.m.functions[0].blocks:
        il = bbl.instructions
        i = 0
        while i < len(il):
            inst = il[i]
            si = inst.sync_info
            limit = 1
            if si is not None and len(si.on_wait) > limit:
                waits = list(si.on_wait)
                extra = waits[limit:]
                for j in range(0, len(extra), 1):
                    nop = mybir.InstNoOp(name=f"I-wsplit-{ctr}", ins=[], outs=[])
                    ctr += 1
                    nop.engine = inst.engine
                    nop.sync_info = bass_rust.SyncInfo(
                        on_wait=[extra[j]], on_update=[]
                    )
                    il.insert(i, nop)
                    i += 1
                si.on_wait = waits[:limit]
                inst.sync_info = si
            i += 1
    return ctr


def _get_program(with_qk_bias=False):
    key = ("nc", with_qk_bias)
    if key not in _CACHE:
        _CACHE[key] = _build_program(with_qk_bias)
    return _CACHE[key]


def _get_program_hw(with_qk_bias=False):
    """Program with the walrus sync-wait workaround applied (breaks CoreSim's
    race detector, so only applied for hardware runs)."""
    nc = _get_program(with_qk_bias)
    skey = ("split_done", with_qk_bias)
    if not _CACHE.get(skey):
        _split_excess_waits(nc)
        _CACHE[skey] = True
    return nc


def _make_in_maps(x1, x2, Wq, bq, Wk, bk, Wv, bv, gamma):
    g = float(np.asarray(gamma).reshape(-1)[0])
    bq = np.asarray(bq, dtype=np.float32).reshape(-1)
    bk = np.asarray(bk, dtype=np.float32).reshape(-1)
    with_qk_bias = bool(np.any(bq)) or bool(np.any(bk))
    bf = ml_dtypes.bfloat16
    shared = {
        "wqT": np.ascontiguousarray(Wq.T).astype(bf),
        "wkT": np.ascontiguousarray(Wk.T).astype(bf),
        "wvT": np.ascontiguousarray(Wv.T).astype(bf),
        "gbv": (g * np.asarray(bv, dtype=np.float32)).reshape(C, 1),
        "gam": np.full((P, 1), g, dtype=np.float32),
    }
    in_maps = []
    for d in range(2):
        src_q, src_kv = (x1, x2) if d == 0 else (x2, x1)
        for b in range(B):
            xq_f = np.ascontiguousarray(src_q[b].reshape(C, N), dtype=np.float32)
            xkv_f = np.ascontiguousarray(src_kv[b].reshape(C, N), dtype=np.float32)
            m = {
                "xqb": xq_f.astype(bf),
                "xkvb": xkv_f.astype(bf),
                "x2tb": np.ascontiguousarray(xkv_f.T).astype(bf),
                **shared,
            }
            if with_qk_bias:
                # per-m softmax term a_m = bq.(Wk xkv)_m; the per-n terms
                # (bk.q + bq.bk) cancel between numerator and denominator
                k_raw = Wk.astype(np.float64) @ xkv_f.astype(np.float64)
                a = (bq.astype(np.float64) @ k_raw).astype(np.float32)
                m["a32"] = np.ascontiguousarray(a.reshape(N_MB, P).T)
            in_maps.append(m)
    return in_maps, with_qk_bias


def kernel(x1, x2, Wq, bq, Wk, bk, Wv, bv, gamma, _want_results=False):
    x1 = np.asarray(x1, dtype=np.float32)
    x2 = np.asarray(x2, dtype=np.float32)
    in_maps, with_qk_bias = _make_in_maps(x1, x2, Wq, bq, Wk, bk, Wv, bv, gamma)
    nc = _get_program_hw(with_qk_bias)

    from concourse.bass_utils import run_bass_kernel_spmd

    res = run_bass_kernel_spmd(nc, in_maps, core_ids=list(range(2 * B)))
    outs = [r["out"].reshape(C, 64, 64) for r in res.results]
    out1 = np.stack(outs[:B]).astype(np.float32)
    out2 = np.stack(outs[B:]).astype(np.float32)
    if _want_results:
        return (out1, out2), res
    return (out1, out2)
